# revision 12
# baseline (speedup 1.0000x reference)
"""DGCNN (4x SAGEConv + SortPool + Conv1d + MLP) Trainium2 Bass kernel.

Sharding: data-parallel over the B=512 graphs -> 64 graphs per core on 8 cores.
Edges never cross graphs, so each core's message passing is local. The edge
list is converted on the host into per-graph integer count matrices
(C[g][s,d] = multiplicity(s->d)); aggregation is a block-diagonal dense matmul
(2 graphs of 64 nodes per 128-partition tile) followed by an on-chip 1/deg
column scale.

Precision scheme: the SortPool argsort keys (feature 255 of layer 3) need
~fp32-exact arithmetic (adjacent sorted keys can differ by ~6e-7; a rank swap
costs ~1e-1 rel error). PE fp32 matmuls run at 4 cyc/row; fp16 runs at
1 cyc/row and the PE multiplies fp16 subnormals exactly. So every fp32 value
is kept as an exact hi+lo fp16 pair (hi=fp16(x), lo=fp16(x-hi), ~22 mantissa
bits) and matmuls are computed as 2-3 fp16 products accumulated in fp32 PSUM:
  agg:    (h_hi + h_lo) @ C         (C integer, fp16-exact: 2 products)
  stage2: W_hi X_hi + W_hi X_lo + W_lo X_hi   (3 products, ~2^-22 accurate)
Measured on-device: this is as accurate as the PE's own fp32 path (1.8e-7 vs
2.2e-7 rel), verified to reproduce the reference key order on all 512 graphs.
Value-only paths (selection, conv, lin1) are plain fp16.

SortPool is exact (stable argsort incl. ties) via a rank computation
  rank(i) = #{j : k_j > k_i}   on keys perturbed by  k_i -= i*1e-11,
with keys extracted in exact fp32 straight from the layer-3 PSUM (row 127 of
the oh=1 chunks). Selection of the top-30 rows per graph is a one-hot matmul.
"""

import numpy as np

import concourse.bass as bass
import concourse.bacc as bacc
import concourse.mybir as mybir
import concourse.tile as tile
from concourse.bass_utils import run_bass_kernel_spmd

B, P, K, KS = 512, 64, 30, 4
N, E, F, H = B * P, 524288, 128, 256
L_OUT = K - KS + 1          # 27
N_CLASSES = 10
N_CORES = 8
GPC = B // N_CORES          # 64 graphs / core
NPC = GPC * P               # 4096 nodes / core
PAIRS = GPC // 2            # 32 pair-tiles (2 graphs of 64 nodes = 128 partitions)
NCHUNK = 512                # free-dim chunk for stage2 matmuls
G4 = 4                      # pairs per PSUM bank group (4*128 = 512 cols)
F32 = mybir.dt.float32
F16 = mybir.dt.float16
EPS_TIE = 1e-11

NLAYERS = 4
GCHUNK = 16                 # graphs per conv psum tile (16*28 = 448)
L28 = L_OUT + 1             # conv free dim padded even
TKPAD = GPC * K + 8         # topkT free size incl. zeroed overrun pad
S1 = 2 * L_OUT              # 54 lin1 contraction steps of 128


def _split16(x):
    hi = x.astype(np.float16)
    lo = (x - hi.astype(np.float32)).astype(np.float16)
    return hi, lo


# ---------------------------------------------------------------- host prep

def _prep_shared(inp):
    """Host-side weight/constant reshaping (identical for every core)."""
    sh = {}
    for li in range(4):
        for nm in ("wl", "wr"):
            w = np.ascontiguousarray(inp[f"sage{li}_{nm}"], np.float32)
            hi, lo = _split16(w)
            sh[f"{nm}{li}h"] = hi
            sh[f"{nm}{li}l"] = lo
        sh[f"b{li}"] = np.ascontiguousarray(inp[f"sage{li}_b"], np.float32)
    w = np.asarray(inp["conv1d_w"], np.float32)            # [O=256, I=256, KS]
    w2 = np.empty((2 * KS, 128, H), np.float16)
    for k in range(KS):
        wt = w[:, :, k].T                                  # [I, O]
        for ih in range(2):
            w2[k * 2 + ih] = wt[ih * 128:(ih + 1) * 128].astype(np.float16)
    sh["w2"] = w2
    sh["cb"] = np.ascontiguousarray(inp["conv1d_b"], np.float32)
    w1 = np.asarray(inp["lin1_w"], np.float32)             # [6912, 256]
    sh["w1"] = np.ascontiguousarray(
        w1.reshape(2, 128, L_OUT, H).transpose(0, 2, 1, 3)
        .reshape(S1, 128, H)).astype(np.float16)
    sh["lb1"] = np.ascontiguousarray(
        np.broadcast_to(np.asarray(inp["lin1_b"], np.float32), (GPC, H)))
    sh["w4"] = np.ascontiguousarray(inp["lin2_w"], np.float32)   # [256, 128]
    sh["b2q"] = np.ascontiguousarray(inp["lin2_b"], np.float32)  # [128]
    sh["w5"] = np.ascontiguousarray(inp["out_w"], np.float32)    # [128, 10]
    sh["b3q"] = np.asarray(inp["out_b"], np.float32).reshape(N_CLASSES, 1).copy()
    sh["iota60"] = np.ascontiguousarray(
        np.broadcast_to(np.arange(2 * K, dtype=np.float32), (128, 2 * K)))
    off30 = np.zeros((128, 1), np.float32)
    off30[64:] = float(K)
    sh["off30"] = off30
    sh["epsrow"] = np.ascontiguousarray(
        np.broadcast_to(np.arange(P, dtype=np.float32) * np.float32(EPS_TIE), (P, P))).astype(np.float32)
    sh["id128"] = np.eye(128, dtype=np.float32)
    sh["id16"] = np.eye(128, dtype=np.float16)
    return sh


def _prep_cores(inp):
    """Per-core shards: split node features and block-diag count adjacency."""
    x = np.nan_to_num(np.asarray(inp["x"], np.float32))
    ei = np.asarray(inp["edge_index"])
    src = ei[0].astype(np.int64)
    dst = ei[1].astype(np.int64)
    deg = np.bincount(dst, minlength=N).astype(np.float32)
    inv_deg = (1.0 / np.maximum(deg, 1.0)).astype(np.float32)
    g = src // P
    flat = g * (P * P) + (src % P) * P + (dst % P)
    C = np.bincount(flat, minlength=B * P * P).astype(np.float16).reshape(B, P, P)

    cores = []
    for c in range(N_CORES):
        xc = np.ascontiguousarray(x[c * NPC:(c + 1) * NPC])          # [4096, 128]
        xh, xl = _split16(xc)
        cbd = np.zeros((PAIRS, 128, 128), np.float16)
        for t in range(PAIRS):
            cbd[t, :P, :P] = C[c * GPC + 2 * t]
            cbd[t, P:, P:] = C[c * GPC + 2 * t + 1]
        cores.append({
            "xh": xh, "xl": xl,
            "xth": np.ascontiguousarray(xh.T),                       # [128, 4096]
            "xtl": np.ascontiguousarray(xl.T),
            "cbd": cbd,
            "invdeg": np.ascontiguousarray(np.broadcast_to(
                inv_deg[c * NPC:(c + 1) * NPC], (128, NPC))),
        })
    return cores


# ---------------------------------------------------------------- device kernel

def _build(nc):
    dt = nc.dram_tensor
    d_xh = dt("xh", [NPC, F], F16, kind="ExternalInput")
    d_xl = dt("xl", [NPC, F], F16, kind="ExternalInput")
    d_xth = dt("xth", [F, NPC], F16, kind="ExternalInput")
    d_xtl = dt("xtl", [F, NPC], F16, kind="ExternalInput")
    d_cbd = dt("cbd", [PAIRS, 128, 128], F16, kind="ExternalInput")
    d_invdeg = dt("invdeg", [128, NPC], F32, kind="ExternalInput")
    d_w = {}
    for li in range(4):
        fin = F if li == 0 else H
        for nm in ("wl", "wr"):
            for hl in "hl":
                d_w[f"{nm}{li}{hl}"] = dt(f"{nm}{li}{hl}", [fin, H], F16,
                                          kind="ExternalInput")
    d_b = [dt(f"b{li}", [H], F32, kind="ExternalInput") for li in range(4)]
    d_w2 = dt("w2", [2 * KS, 128, H], F16, kind="ExternalInput")
    d_cb = dt("cb", [H], F32, kind="ExternalInput")
    d_w1 = dt("w1", [S1, 128, H], F16, kind="ExternalInput")
    d_lb1 = dt("lb1", [GPC, H], F32, kind="ExternalInput")
    d_w4 = dt("w4", [H, 128], F32, kind="ExternalInput")
    d_b2q = dt("b2q", [128], F32, kind="ExternalInput")
    d_w5 = dt("w5", [128, N_CLASSES], F32, kind="ExternalInput")
    d_b3q = dt("b3q", [N_CLASSES, 1], F32, kind="ExternalInput")
    d_iota60 = dt("iota60", [128, 2 * K], F32, kind="ExternalInput")
    d_off30 = dt("off30", [128, 1], F32, kind="ExternalInput")
    d_epsrow = dt("epsrow", [P, P], F32, kind="ExternalInput")
    d_id128 = dt("id128", [128, 128], F32, kind="ExternalInput")
    d_id16 = dt("id16", [128, 128], F16, kind="ExternalInput")
    d_out = dt("out", [GPC, N_CLASSES], F32, kind="ExternalOutput")

    with tile.TileContext(nc) as tc:
        _emit(tc, nc, locals())
    nc.compile()
    return nc


def _ap(base, extra_offset, free_dims):
    """Build a custom AP view: keep base's partition dim, replace free dims."""
    return bass.AP(base.tensor, base.offset + extra_offset,
                   [base.ap[0]] + list(free_dims))


def _emit(tc, nc, d):
    from contextlib import ExitStack
    ctx = ExitStack()
    with ctx:
        persist = ctx.enter_context(tc.tile_pool(name="persist", bufs=1))
        act_pool = ctx.enter_context(tc.tile_pool(name="acts", bufs=1))

        # ---- persistent loads (weight DMAs deferred until after input DMAs)
        _deferred = []

        def load(name, shape, view=None, dram=None, dtype=F32):
            t = persist.tile(shape, dtype, tag=name)
            src = (dram if dram is not None else d[f"d_{name}"]).ap()
            if view is not None:
                src = src.rearrange(*view[0], **view[1])
            _deferred.append((t, src))
            return t

        wlh, wll, wrh, wrl, bias = [], [], [], [], []
        for li in range(4):
            ki = 1 if li == 0 else 2
            vw = (["(k p) o -> p k o"], {"p": 128})
            wlh.append(load(f"wl{li}h", [128, ki, H], vw, dram=d["d_w"][f"wl{li}h"],
                            dtype=F16))
            wll.append(load(f"wl{li}l", [128, ki, H], vw, dram=d["d_w"][f"wl{li}l"],
                            dtype=F16))
            wrh.append(load(f"wr{li}h", [128, ki, H], vw, dram=d["d_w"][f"wr{li}h"],
                            dtype=F16))
            wrl.append(load(f"wr{li}l", [128, ki, H], vw, dram=d["d_w"][f"wr{li}l"],
                            dtype=F16))
            bias.append(load(f"b{li}", [128, 2], (["(h p) -> p h"], {"p": 128}),
                             dram=d["d_b"][li]))
        w2 = load("w2", [128, 2 * KS, H], (["k p o -> p k o"], {}), dtype=F16)
        cb = load("cb", [128, 2], (["(h p) -> p h"], {"p": 128}))
        b1 = load("lb1", [GPC, H])
        w4 = load("w4", [128, 2, 128], (["(k p) o -> p k o"], {"p": 128}))
        b2q = load("b2q", [128, 1])
        w5 = load("w5", [128, N_CLASSES])
        b3q = load("b3q", [N_CLASSES, 1])
        iota60 = load("iota60", [128, 2 * K])
        off30 = load("off30", [128, 1])
        epsrow = load("epsrow", [P, P])
        id128 = load("id128", [128, 128])
        id16 = load("id16", [128, 128], dtype=F16)
        invdeg = load("invdeg", [128, NPC])

        # ---- long-lived activations
        h_hi = act_pool.tile([128, PAIRS, H], F16, tag="hhi")    # nodes on partitions
        h_lo = act_pool.tile([128, PAIRS, H], F16, tag="hlo")
        k16h = act_pool.tile([P, P], F16, tag="k16h")            # key rows (hi/lo)
        k16l = act_pool.tile([P, P], F16, tag="k16l")

        qs = [nc.sync, nc.scalar, nc.gpsimd]

        with tc.tile_pool(name="sage", bufs=1) as sg:
            # hT ping-pong (hi/lo pairs); aggT split
            hts_hi = [sg.tile([128, 2, NPC], F16, tag=f"hT{i}h", name=f"hT{i}h")
                      for i in range(2)]
            hts_lo = [sg.tile([128, 2, NPC], F16, tag=f"hT{i}l", name=f"hT{i}l")
                      for i in range(2)]
            aggT_hi = sg.tile([128, 2, NPC], F16, tag="aggh")
            aggT_lo = sg.tile([128, 2, NPC], F16, tag="aggl")

            cbd_parts = []
            for g in range(4):
                sl8g = slice(g * 8, (g + 1) * 8)
                sl8 = (slice(None), sl8g, slice(None))
                # x lands directly in h_hi/h_lo[:, :, 0:128] (layer-0 features)
                nc.sync.dma_start(
                    h_hi[:, sl8g, 0:F],
                    d["d_xh"].ap().rearrange("(t p) f -> p t f", p=128)[sl8])
                nc.scalar.dma_start(
                    h_lo[:, sl8g, 0:F],
                    d["d_xl"].ap().rearrange("(t p) f -> p t f", p=128)[sl8])
                t_cb = persist.tile([128, PAIRS // 4, 128], F16, tag=f"cbd{g}",
                                    name=f"cbd{g}")
                nc.gpsimd.dma_start(
                    t_cb[...], d["d_cbd"].ap().rearrange("t p n -> p t n")[sl8])
                nc.gpsimd.dma_start(
                    invdeg[:, g * 1024:(g + 1) * 1024],
                    d["d_invdeg"].ap()[:, g * 1024:(g + 1) * 1024])
                cbd_parts.append(t_cb)
            # remove invdeg from deferred (loaded above)
            _deferred[:] = [(t, s) for (t, s) in _deferred if t is not invdeg]
            for g in range(4):
                nc.sync.dma_start(hts_hi[0][:, 0, g * 1024:(g + 1) * 1024],
                                  d["d_xth"].ap()[:, g * 1024:(g + 1) * 1024])
                nc.scalar.dma_start(hts_lo[0][:, 0, g * 1024:(g + 1) * 1024],
                                    d["d_xtl"].ap()[:, g * 1024:(g + 1) * 1024])
            for i, (t, srcap) in enumerate(_deferred):
                qs[i % 3].dma_start(t[...], srcap)
            _deferred.clear()

            with tc.tile_pool(name="scr", bufs=3) as scr, \
                 tc.tile_pool(name="ps_a", bufs=2, space="PSUM") as psa_p, \
                 tc.tile_pool(name="ps_w", bufs=2, space="PSUM") as psw_p, \
                 tc.tile_pool(name="ps_t", bufs=2, space="PSUM") as pst_p:
                for li in range(NLAYERS):
                    ki = 1 if li == 0 else 2
                    last = li == NLAYERS - 1
                    hTv_hi, hTv_lo = hts_hi[li % 2], hts_lo[li % 2]
                    hTo_hi, hTo_lo = hts_hi[(li + 1) % 2], hts_lo[(li + 1) % 2]

                    # ---- agg: aggT_raw[f, n'] = sum_n h[n, f] C[n, n'] per pair,
                    # then column-scale by 1/deg and split to fp16 hi/lo.
                    for g4 in range(PAIRS // G4):
                        seg = slice(g4 * G4 * 128, (g4 + 1) * G4 * 128)
                        for mh in range(ki):
                            ps = psa_p.tile([128, G4, 128], F32, tag="psa")
                            for i in range(G4):
                                t = g4 * G4 + i
                                lh = h_hi[:, t, mh * 128:(mh + 1) * 128]
                                ll = h_lo[:, t, mh * 128:(mh + 1) * 128]
                                rhs = cbd_parts[t // 8][:, t % 8, :]
                                nc.tensor.matmul(ps[:, i, :], lhsT=lh, rhs=rhs,
                                                 start=True, stop=False)
                                nc.tensor.matmul(ps[:, i, :], lhsT=ll, rhs=rhs,
                                                 start=False, stop=True)
                            t32 = scr.tile([128, NCHUNK], F32, tag="t32")
                            nc.vector.tensor_tensor(
                                t32[...], ps[...].rearrange("p a b -> p (a b)"),
                                invdeg[:, seg], op=mybir.AluOpType.mult)
                            nc.gpsimd.tensor_copy(aggT_hi[:, mh, seg], t32[...])
                            nc.gpsimd.tensor_tensor(
                                aggT_lo[:, mh, seg], t32[...], aggT_hi[:, mh, seg],
                                op=mybir.AluOpType.subtract)

                    # ---- stage2: hT_next[o, n] = relu(wl.agg + wr.h + b) via
                    # 3-product fp16 splits accumulated in fp32 PSUM.
                    for oh in range(2):
                        osl = slice(oh * 128, (oh + 1) * 128)
                        for ncki in range(NPC // NCHUNK):
                            sl = slice(ncki * NCHUNK, (ncki + 1) * NCHUNK)
                            ps = psw_p.tile([128, NCHUNK], F32, tag="psw")
                            step, nsteps = 0, 2 * ki * 3
                            for wh, wl_, xh_, xl_ in (
                                    (wlh[li], wll[li], aggT_hi, aggT_lo),
                                    (wrh[li], wrl[li], hTv_hi, hTv_lo)):
                                for kh in range(ki):
                                    for lt, rt in ((wh, xh_), (wh, xl_), (wl_, xh_)):
                                        nc.tensor.matmul(
                                            ps[...], lhsT=lt[:, kh, osl],
                                            rhs=rt[:, kh, sl],
                                            start=(step == 0), stop=(step == nsteps - 1))
                                        step += 1
                            if last and oh == 0:
                                # layer-3 oh=0 feeds only the (fp16) value path
                                nc.scalar.activation(
                                    hTo_hi[:, oh, sl], ps[...],
                                    mybir.ActivationFunctionType.Relu,
                                    bias=bias[li][:, oh:oh + 1])
                            else:
                                h32 = scr.tile([128, NCHUNK], F32, tag="h32")
                                nc.vector.tensor_scalar(
                                    h32[...], ps[...], bias[li][:, oh:oh + 1], 0.0,
                                    op0=mybir.AluOpType.add, op1=mybir.AluOpType.max)
                                nc.scalar.activation(
                                    hTo_hi[:, oh, sl], h32[...],
                                    mybir.ActivationFunctionType.Copy, bias=0.0)
                                nc.gpsimd.tensor_tensor(
                                    hTo_lo[:, oh, sl], h32[...], hTo_hi[:, oh, sl],
                                    op=mybir.AluOpType.subtract)

                    # ---- h_next = transpose(hT_next) per pair (fp16 PE transpose)
                    for g4 in range(PAIRS // G4):
                        for oh in range(2):
                            srcs = [(hTo_hi, h_hi)] if last else \
                                [(hTo_hi, h_hi), (hTo_lo, h_lo)]
                            for src, dst_t in srcs:
                                ps = pst_p.tile([128, G4, 128], F16, tag="pst")
                                for i in range(G4):
                                    t = g4 * G4 + i
                                    nc.tensor.transpose(
                                        ps[:, i, :],
                                        src[:, oh, t * 128:(t + 1) * 128], id16[...])
                                eng = nc.vector if (g4 + oh) % 2 == 0 else nc.any
                                eng.tensor_copy(
                                    dst_t[:, g4 * G4:(g4 + 1) * G4,
                                          oh * 128:(oh + 1) * 128], ps[...])

            # exact sort keys: fp16 hi+lo rows of feature 255 of layer 3
            hT3_hi, hT3_lo = hts_hi[NLAYERS % 2], hts_lo[NLAYERS % 2]
            nc.sync.dma_start(k16h[...], hT3_hi[127:128, 1, :])
            nc.scalar.dma_start(k16l[...], hT3_lo[127:128, 1, :])

        # ---------------- sort: ranks of the exact keys per graph
        with tc.tile_pool(name="sort", bufs=1) as sp:
            rt = sp.tile([P, P], F32, tag="rt")
            pt_all = sp.tile([128, PAIRS, 2 * K], F16, tag="pt")
            with tc.tile_pool(name="sort_scratch", bufs=1) as ss:
                # exact keys: recombine fp16 hi+lo rows in fp32
                km = ss.tile([P, P], F32, tag="km")
                nc.vector.tensor_tensor(km[...], k16h[...], k16l[...],
                                        op=mybir.AluOpType.add)
                kmp = ss.tile([P, P], F32, tag="kmp")
                nc.vector.tensor_sub(kmp[...], km[...], epsrow[...])
                cbt = ss.tile([P, P * P], F32, tag="cbt")
                kb = kmp[:, :]
                in0 = _ap(kb, 0, [[0, P], kb.ap[1]])       # [g, i(bc), j]   k(g, j)
                in1 = _ap(kb, 0, [kb.ap[1], [0, P]])       # [g, i, j(bc)]   k(g, i)
                nc.vector.tensor_tensor(
                    _ap(cbt[:, :], 0, [[P, P], [1, P]]), in0, in1,
                    op=mybir.AluOpType.is_gt)
                rk = ss.tile([P, P], F32, tag="rk")
                nc.vector.tensor_reduce(
                    rk[...], _ap(cbt[:, :], 0, [[P, P], [1, P]]),
                    axis=mybir.AxisListType.X, op=mybir.AluOpType.add)
                with tc.tile_pool(name="ps_sort", bufs=1, space="PSUM") as pss:
                    pr = pss.tile([P, P], F32, tag="pr")
                    nc.tensor.transpose(pr[...], rk[...], id128[0:P, 0:P])
                    nc.any.tensor_copy(rt[...], pr[...])
                # rankP[p, t] = rank(node p%64 of graph 2t + p//64)
                rankp = ss.tile([128, PAIRS], F32, tag="rankp")
                rb = rt[:, :]
                nc.vector.tensor_copy(rankp[0:P, :], _ap(rb, 0, [[2, PAIRS]]))
                nc.sync.dma_start(rankp[P:128, :], _ap(rb, 1, [[2, PAIRS]]))
                # rank2 = rankp + 30*(p>=64) + 1000*(rankp>=30)
                ge30 = ss.tile([128, PAIRS], F32, tag="ge30")
                nc.vector.tensor_scalar(ge30[...], rankp[...], float(K), None,
                                        op0=mybir.AluOpType.is_ge)
                rank2 = ss.tile([128, PAIRS], F32, tag="rank2")
                nc.vector.scalar_tensor_tensor(rank2[...], ge30[...], 1000.0,
                                               rankp[...], op0=mybir.AluOpType.mult,
                                               op1=mybir.AluOpType.add)
                nc.vector.tensor_scalar(rank2[...], rank2[...], off30[:, 0:1], None,
                                        op0=mybir.AluOpType.add)
                # one-hot selection matrices  PT[p, t, c] = (c == rank2[p, t])
                io = iota60[:, :]
                r2 = rank2[:, :]
                nc.vector.tensor_tensor(
                    pt_all[...],
                    _ap(io, 0, [[0, PAIRS], [1, 2 * K]]),
                    _ap(r2, 0, [[1, PAIRS], [0, 2 * K]]),
                    op=mybir.AluOpType.is_equal)

            # ---------------- selection + conv + mlp
            with tc.tile_pool(name="tail", bufs=1) as tp, \
                 tc.tile_pool(name="w1s", bufs=S1) as w1p, \
                 tc.tile_pool(name="ps_tail", bufs=2, space="PSUM") as ptl, \
                 tc.tile_pool(name="ps_fin", bufs=1, space="PSUM") as pfin:
                # prefetch all lin1 weight tiles (fp16, 3.5 MB)
                w1ts = []
                for s in range(S1):
                    w1t = w1p.tile([128, H], F16, tag="w1t")
                    qs[s % 3].dma_start(w1t[...], d["d_w1"].ap()[s])
                    w1ts.append(w1t)

                # topkT[f, b*30+r] = sum_n h_hi[n, f] * PT[n, b(pair), r]
                topkT = tp.tile([128, 2, TKPAD], F16, tag="topkT")
                nc.vector.memset(topkT[:, :, GPC * K:], 0.0)
                for t in range(PAIRS):
                    for mh in range(2):
                        ps = ptl.tile([128, 2 * K], F32, tag="pssel")
                        nc.tensor.matmul(
                            ps[...],
                            lhsT=h_hi[:, t, mh * 128:(mh + 1) * 128],
                            rhs=pt_all[:, t, :],
                            start=True, stop=True)
                        nc.any.tensor_copy(
                            topkT[:, mh, t * 2 * K:(t + 1) * 2 * K], ps[...])

                # conv1d: y[p, oh, b, l] = relu(sum_{k, ih} w2^T topkT[:, b*30+l+k] + cb)
                y_sb = tp.tile([128, 2, GPC, L28], F16, tag="y")
                for oh in range(2):
                    for bc in range(GPC // GCHUNK):
                        ps = ptl.tile([128, GCHUNK, L28], F32, tag="psconv")
                        step = 0
                        for k in range(KS):
                            for ih in range(2):
                                base = topkT[:, ih, :]
                                rhs = _ap(base, bc * GCHUNK * K + k,
                                          [[K, GCHUNK], [1, L28]])
                                nc.tensor.matmul(
                                    ps[...],
                                    lhsT=w2[:, k * 2 + ih, oh * 128:(oh + 1) * 128],
                                    rhs=rhs,
                                    start=(step == 0), stop=(step == 2 * KS - 1))
                                step += 1
                        nc.scalar.activation(
                            y_sb[:, oh, bc * GCHUNK:(bc + 1) * GCHUNK, :], ps[...],
                            mybir.ActivationFunctionType.Relu,
                            bias=cb[:, oh:oh + 1])

                # lin1 (b-major): z1T[b, o] = relu(sum_s y_s^T @ w1_s + b1)
                ps1 = pfin.tile([GPC, H], F32, tag="ps1")
                for s in range(S1):
                    ot, l = divmod(s, L_OUT)
                    nc.tensor.matmul(
                        ps1[...],
                        lhsT=y_sb[:, ot, :, l],
                        rhs=w1ts[s][...],
                        start=(s == 0), stop=(s == S1 - 1))
                z1t = tp.tile([GPC, H], F32, tag="z1t")
                nc.vector.tensor_add(z1t[...], ps1[...], b1[...])
                nc.scalar.activation(z1t[...], z1t[...],
                                     mybir.ActivationFunctionType.Relu, bias=0.0)
                # transpose z1T -> z1 [o on partitions]
                z1 = tp.tile([128, 2, GPC], F32, tag="z1")
                for mh in range(2):
                    psz = pfin.tile([128, GPC], F32, tag="psz")
                    nc.tensor.transpose(psz[...],
                                        z1t[:, mh * 128:(mh + 1) * 128],
                                        id128[0:GPC, 0:GPC])
                    nc.any.tensor_copy(z1[:, mh, :], psz[...])

                # lin2 + out
                ps2 = pfin.tile([128, GPC], F32, tag="ps2")
                for kh in range(2):
                    nc.tensor.matmul(ps2[...], lhsT=w4[:, kh, :], rhs=z1[:, kh, :],
                                     start=(kh == 0), stop=(kh == 1))
                z2 = tp.tile([128, GPC], F32, tag="z2")
                nc.scalar.activation(z2[...], ps2[...],
                                     mybir.ActivationFunctionType.Relu,
                                     bias=b2q[:, 0:1])
                ps3 = pfin.tile([N_CLASSES, GPC], F32, tag="ps3")
                nc.tensor.matmul(ps3[...], lhsT=w5[...], rhs=z2[...],
                                 start=True, stop=True)
                o_sb = tp.tile([N_CLASSES, GPC], F32, tag="osb")
                nc.scalar.activation(o_sb[...], ps3[...],
                                     mybir.ActivationFunctionType.Relu,
                                     bias=b3q[:, 0:1])
                nc.sync.dma_start(d["d_out"].ap().rearrange("b o -> o b"), o_sb[...])


# ---------------------------------------------------------------- entry point

_CACHED = {}


def _get_nc():
    if "nc" not in _CACHED:
        nc = bacc.Bacc("TRN2", target_bir_lowering=False, debug=False,
                       enable_asserts=True)
        _CACHED["nc"] = _build(nc)
    return _CACHED["nc"]


def make_in_maps(inputs):
    sh = _prep_shared(inputs)
    cores = _prep_cores(inputs)
    return [{**sh, **c} for c in cores]


TRACE = False


def kernel(**inputs):
    in_maps = make_in_maps(inputs)
    nc = _get_nc()
    res = run_bass_kernel_spmd(nc, in_maps, core_ids=list(range(N_CORES)),
                               trace=TRACE)
    _CACHED["last_res"] = res
    return np.concatenate([r["out"] for r in res.results], axis=0)


if __name__ == "__main__":
    import reference
    inputs = {k: np.asarray(v) for k, v in reference.setup_inputs().items()}
    out = kernel(**inputs)
    print("out", out.shape, out.dtype)
